# revision 32
# baseline (speedup 1.0000x reference)
"""Multi-head attention (B=4, P=2048, D=1024, H=16) on 8 TRN2 NeuronCores.

Sharding: tensor-parallel over heads (2 heads per core). Each core computes
qkv for its heads, full attention for its heads, and a partial output
projection (rows of w_proj for its heads). Partials are summed on host.

v6: ACT-streaming pipeline. The exp activations (342us of Scalar work) are
the engine floor, so the kernel is one flat 16-sweep x 16-keyblock pipeline
where the Scalar engine never idles: score matmuls run one step ahead of
exp, attention-value matmuls one step behind. Score pairs are row-packed
(head0 on PE rows 0-63, head1 on 64-127, concurrent via tile_position);
attn-value pairs are col-packed (head0 -> PSUM partitions 0-63, head1 ->
64-127), which also lands o^T head-stacked for a single K=128 output
projection matmul. Softmax denominators come from DVE-accumulated exp
tiles + an M=1 ones-matmul + a K=2 broadcast matmul. qkv projection,
v transposes, and output projection are emitted as budgeted PE filler
inside the sweeps. PSUM: 4 banks score double-buffer + 2 attnV + 1 qkv
chain + 1 misc = 8.
"""

import numpy as np
import ml_dtypes

import concourse.bass as bass
import concourse.tile as tile
from concourse import bacc, mybir
from concourse import bass_utils
from concourse.masks import make_identity

B, P, D = 4, 2048, 1024
H = 16
NCORES = 8
HPC = H // NCORES          # heads per core = 2
d = D // H                 # 64
R = B * P                  # 8192
SCALE = float(d) ** -0.5
NS = 16                    # sweeps: (b, ic) with 512 queries each
NJ = 16                    # key blocks of 128 per sweep

F32 = mybir.dt.float32
F32R = mybir.dt.float32r
BF16 = mybir.dt.bfloat16
AF = mybir.ActivationFunctionType

_CACHE = {}
DEBUG_DUMP = False


def _build():
    nc = bacc.Bacc("TRN2", target_bir_lowering=False, debug=False,
                   enable_asserts=False)
    xT = nc.dram_tensor("xT", (D, R), BF16, kind="ExternalInput").ap()
    wqkv = nc.dram_tensor("wqkv", (128, 3072), BF16, kind="ExternalInput").ap()
    wproj = nc.dram_tensor("wproj", (128, D), BF16, kind="ExternalInput").ap()
    out = nc.dram_tensor("out", (R, D), F32, kind="ExternalOutput").ap()
    if DEBUG_DUMP:
        dbg = {
            "qt0": nc.dram_tensor("d_qt0", (128, P), BF16,
                                  kind="ExternalOutput").ap(),
            "kt0": nc.dram_tensor("d_kt0", (128, P), BF16,
                                  kind="ExternalOutput").ap(),
            "v20": nc.dram_tensor("d_v20", (128, NJ * 128), BF16,
                                  kind="ExternalOutput").ap(),
            "es0": nc.dram_tensor("d_es0", (128, 512), BF16,
                                  kind="ExternalOutput").ap(),
            "es1": nc.dram_tensor("d_es1", (128, 512), BF16,
                                  kind="ExternalOutput").ap(),
            "otn0": nc.dram_tensor("d_otn0", (128, 512), BF16,
                                   kind="ExternalOutput").ap(),
            "rbs0": nc.dram_tensor("d_rbs0", (128, 512), F32,
                                   kind="ExternalOutput").ap(),
            "e00": nc.dram_tensor("d_e00", (128, 1024), BF16,
                                  kind="ExternalOutput").ap(),
        }

    xT3 = xT.rearrange("(kb p) n -> p kb n", p=128)      # [128, 8, 8192]
    out3 = out.rearrange("(r p) n -> p r n", p=128)      # [128, 64, 1024]

    with tile.TileContext(nc) as tc:
        from contextlib import ExitStack
        from collections import deque
        with ExitStack() as ctx:
            p_const = ctx.enter_context(tc.tile_pool(name="const", bufs=1))
            p_w = ctx.enter_context(tc.tile_pool(name="w", bufs=1))
            p_x = ctx.enter_context(tc.tile_pool(name="x", bufs=6))
            p_qk = ctx.enter_context(tc.tile_pool(name="qk", bufs=2))
            p_vt = ctx.enter_context(tc.tile_pool(name="vt", bufs=2))
            p_e = ctx.enter_context(tc.tile_pool(name="e", bufs=12))
            p_es = ctx.enter_context(tc.tile_pool(name="es", bufs=2))
            p_ot = ctx.enter_context(tc.tile_pool(name="ot", bufs=3))
            p_nrm = ctx.enter_context(tc.tile_pool(name="nrm", bufs=2))
            p_out = ctx.enter_context(tc.tile_pool(name="o", bufs=2))
            # PSUM: 4 + 2 + 1 + 1 = 8 banks
            ps_sc = ctx.enter_context(
                tc.tile_pool(name="pssc", bufs=2, space="PSUM"))
            ps_av = ctx.enter_context(
                tc.tile_pool(name="psav", bufs=2, space="PSUM"))
            ps_ch = ctx.enter_context(
                tc.tile_pool(name="psch", bufs=1, space="PSUM"))
            ps_ms = ctx.enter_context(
                tc.tile_pool(name="psms", bufs=1, space="PSUM"))

            ident = p_const.tile([128, 128], BF16)
            make_identity(nc, ident[:])
            # all-ones stationary: dn = ones^T @ esum broadcasts the
            # per-query denominator to every output partition
            ones_mat = p_const.tile([128, 128], BF16)
            nc.vector.memset(ones_mat[:], 1.0)

            wq_sb = p_w.tile([128, 3072], BF16)
            nc.sync.dma_start(wq_sb[:], wqkv[:])
            wp_sb = p_w.tile([128, D], BF16)
            nc.sync.dma_start(wp_sb[:], wproj[:])

            batches = [dict() for _ in range(B)]
            sweeps = [dict() for _ in range(NS)]
            filler = deque()          # (cost_ns, closure)

            def pop_filler(budget):
                while filler and budget > 0:
                    c, fn = filler.popleft()
                    fn()
                    budget -= c

            # ---------------- stage A (qkv + v transposes) ----------------
            def alloc_xt(b):
                batches[b]["xt"] = [
                    p_x.tile([128, 8 * 512], BF16, tag="xt",
                             name=f"xt{b}_{cc}")
                    for cc in range(4)]

            def emit_xt_piece(b, piece):
                # one per-kb DMA piece; 32 pieces cover the batch. Spreading
                # them keeps the Sync queue from blocking proj out-DMAs.
                cc, kb = divmod(piece, 8)
                c = b * 4 + cc
                xt = batches[b]["xt"][cc]
                nc.sync.dma_start(xt[:, kb * 512:(kb + 1) * 512],
                                  xT3[:, kb, c * 512:(c + 1) * 512])

            def emit_xt_dmas(b, per_kb=False):
                alloc_xt(b)
                for piece in range(32):
                    emit_xt_piece(b, piece)

            def alloc_batch(b):
                bt = batches[b]
                bt["qt"] = p_qk.tile([128, P], BF16, tag="qt", name=f"qt{b}")
                bt["kt"] = p_qk.tile([128, P], BF16, tag="kt", name=f"kt{b}")
                bt["v2"] = p_qk.tile([128, NJ * 128], BF16, tag="v2",
                                     name=f"v2{b}")

            def chain_half(b, cc, m, half, psum_pool, psum_tag):
                """One half (4 kb) of an 8-matmul qkv chain."""
                bt = batches[b]
                if half == 0:
                    ps = psum_pool.tile([128, 512], F32, tag=psum_tag,
                                        name=f"psq{b}_{cc}_{m}")
                    bt[("ps", cc, m)] = ps
                else:
                    ps = bt.pop(("ps", cc, m))
                xt = bt["xt"][cc]
                for kb in range(4 * half, 4 * half + 4):
                    col = kb * 384 + m * 128
                    nc.tensor.matmul(
                        ps[:], wq_sb[:, col:col + 128],
                        xt[:, kb * 512:(kb + 1) * 512],
                        start=(kb == 0), stop=(kb == 7))
                if half == 1:
                    if m == 0:
                        nc.vector.tensor_copy(
                            bt["qt"][:, cc * 512:(cc + 1) * 512], ps[:])
                    elif m == 1:
                        nc.vector.tensor_copy(
                            bt["kt"][:, cc * 512:(cc + 1) * 512], ps[:])
                    else:
                        vtmp = p_vt.tile([128, 512], BF16, tag="vt",
                                         name=f"vtmp{b}_{cc}")
                        nc.vector.tensor_copy(vtmp[:], ps[:])
                        bt[("vtmp", cc)] = vtmp

            def transpose_pair(b, cc, rs, psum_pool, psum_tag):
                """Both heads' v transposes for one key block: two transpose
                matmuls into one psum tile, one copy out."""
                bt = batches[b]
                vtmp = bt[("vtmp", cc)]
                jb = cc * 4 + rs
                for h in range(2):
                    pt = psum_pool.tile([128, 64], BF16, tag=psum_tag,
                                        name=f"pt{b}_{jb}_{h}")
                    nc.tensor.transpose(
                        pt[:],
                        vtmp[h * 64:(h + 1) * 64, rs * 128:(rs + 1) * 128],
                        ident[h * 64:(h + 1) * 64, h * 64:(h + 1) * 64])
                    nc.vector.tensor_copy(
                        bt["v2"][:, jb * 128 + h * 64:jb * 128 + (h + 1) * 64],
                        pt[:])

            def make_stage_a_units(b):
                units = []
                units.append((0, lambda b=b: alloc_batch(b)))
                for cc in range(4):
                    for m in range(3):
                        for half in range(2):
                            units.append(
                                (850, lambda b=b, cc=cc, m=m, half=half:
                                 chain_half(b, cc, m, half, ps_ch, "ch")))
                    for rs in range(4):
                        units.append(
                            (330, lambda b=b, cc=cc, rs=rs:
                             transpose_pair(b, cc, rs, ps_ms, "ms")))
                return units

            # ---------------- attention sweep pieces ----------------
            def emit_score_pair(s, jb):
                b = s // 4
                ic = s % 4
                bt = batches[b]
                sc = ps_sc.tile([128, 1024], F32, tag="sc",
                                name=f"sc{s}_{jb}")
                for h in range(2):
                    nc.tensor.matmul(
                        sc[:, h * 512:(h + 1) * 512],
                        bt["kt"][h * 64:(h + 1) * 64,
                                 jb * 128:(jb + 1) * 128],
                        bt["qt"][h * 64:(h + 1) * 64,
                                 ic * 512:(ic + 1) * 512],
                        start=True, stop=True)
                sweeps[s][("sc", jb)] = sc

            def emit_exp(s, jb):
                sc = sweeps[s].pop(("sc", jb))
                et = p_e.tile([128, 1024], BF16, tag="e", name=f"e{s}_{jb}")
                nc.scalar.activation(et[:], sc[:], AF.Exp, scale=SCALE)
                if DEBUG_DUMP and s == 0 and jb == 0:
                    nc.sync.dma_start(dbg["e00"][:], et[:])
                sweeps[s][("e", jb)] = et

            def emit_attnv(s, jb):
                st = sweeps[s]
                b = s // 4
                et = st[("e", jb)]
                if jb == 0:
                    st["av"] = ps_av.tile([128, 512], F32, tag="av",
                                          name=f"av{s}")
                av = st["av"]
                v2 = batches[b]["v2"]
                for h in range(2):
                    nc.tensor.matmul(
                        av[h * 64:(h + 1) * 64, :],
                        v2[:, jb * 128 + h * 64:jb * 128 + (h + 1) * 64],
                        et[:, h * 512:(h + 1) * 512],
                        start=(jb == 0), stop=(jb == NJ - 1))

            def emit_esum(s, jb):
                # single [128, 1024] op covering both heads amortizes the
                # DVE fixed cost (the adds already run in 2x_1p mode)
                st = sweeps[s]
                et = st.pop(("e", jb))
                if jb == 0:
                    es = p_es.tile([128, 1024], BF16, tag="es",
                                   name=f"es{s}")
                    nc.vector.tensor_copy(es[:], et[:])
                    st["es"] = es
                else:
                    es = st["es"]
                    nc.vector.tensor_add(es[:], es[:], et[:])

            def emit_norm_a(s):
                st = sweeps[s]
                dn = ps_ms.tile([128, 512], F32, tag="ms", name=f"dn{s}")
                # col-packed pair: each head's denominator broadcast to its
                # 64 output partitions (M=64; dst base 0 / 64)
                for h in range(2):
                    nc.tensor.matmul(dn[h * 64:(h + 1) * 64, :],
                                     ones_mat[:, 0:64],
                                     st["es"][:, h * 512:(h + 1) * 512],
                                     start=True, stop=True)
                rbs = p_nrm.tile([128, 512], F32, tag="rb", name=f"rbs{s}")
                nc.vector.reciprocal_approx_fast(rbs[:], dn[:])
                st["rbs"] = rbs

            def emit_norm_b(s):
                st = sweeps[s]
                rbs = st.pop("rbs")
                if DEBUG_DUMP and s == 0:
                    nc.sync.dma_start(dbg["es0"][:], st["es"][:, 0:512])
                    nc.sync.dma_start(dbg["es1"][:], st["es"][:, 512:1024])
                    nc.sync.dma_start(dbg["rbs0"][:], rbs[:])
                oTn = p_ot.tile([128, 512], BF16, tag="otn", name=f"oTn{s}")
                av = st.pop("av")
                nc.vector.tensor_mul(oTn[:], av[:], rbs[:])
                if DEBUG_DUMP and s == 0:
                    nc.sync.dma_start(dbg["otn0"][:], oTn[:])
                st.pop("es")
                st["oTn"] = oTn
                if s < NS - 1:
                    for rr in range(4):
                        for n2 in range(2):
                            filler.append(
                                (280, lambda s=s, rr=rr, n2=n2:
                                 proj_unit(s, rr, n2)))

            def proj_unit(s, rr, n2, pool=None, tag="ms"):
                st = sweeps[s]
                b = s // 4
                ic = s % 4
                if n2 == 0:
                    st[("os", rr)] = p_out.tile([128, 1024], F32, tag="os",
                                                name=f"os{s}_{rr}")
                outsb = st[("os", rr)]
                psp = (pool or ps_ms).tile([128, 512], F32, tag=tag,
                                           name=f"psp{s}_{rr}_{n2}")
                nc.tensor.matmul(
                    psp[:], st["oTn"][:, rr * 128:(rr + 1) * 128],
                    wp_sb[:, n2 * 512:(n2 + 1) * 512],
                    start=True, stop=True)
                nc.vector.tensor_copy(
                    outsb[:, n2 * 512:(n2 + 1) * 512], psp[:])
                if n2 == 1:
                    st.pop(("os", rr))
                    r0 = b * 16 + ic * 4 + rr
                    nc.sync.dma_start(
                        out3[:, r0:r0 + 1, :],
                        outsb.rearrange("p (r n) -> p r n", n=1024))

            # ---------------- startup: stage A for batch 0 ----------------
            # critical path only: kt for all chunks (scores of sweep 0 need
            # all keys), then qt/v/transposes for chunk 0. The rest becomes
            # filler inside the first sweeps.
            emit_xt_dmas(0, per_kb=True)
            alloc_batch(0)
            for m in (1, 0, 2):
                for half in range(2):
                    chain_half(0, 0, m, half, ps_sc, "sc")
            for rs in range(4):
                transpose_pair(0, 0, rs, ps_sc, "sc")
            # kt for chunks 1-3 first (scores of sweep 0 need keys jb>=4
            # within a few steps), then v/transposes/q per chunk
            for cc in range(1, 4):
                for half in range(2):
                    filler.append(
                        (850, lambda cc=cc, half=half:
                         chain_half(0, cc, 1, half, ps_ch, "ch")))
            for cc in range(1, 4):
                for m in (2, 0):
                    for half in range(2):
                        filler.append(
                            (850, lambda cc=cc, m=m, half=half:
                             chain_half(0, cc, m, half, ps_ch, "ch")))
                    if m == 2:
                        for rs in range(4):
                            filler.append(
                                (330, lambda cc=cc, rs=rs:
                                 transpose_pair(0, cc, rs, ps_ms, "ms")))

            if DEBUG_DUMP:
                nc.sync.dma_start(dbg["qt0"][:], batches[0]["qt"][:])
                nc.sync.dma_start(dbg["kt0"][:], batches[0]["kt"][:])
                nc.sync.dma_start(dbg["v20"][:], batches[0]["v2"][:])

            # schedules: xt DMA emission sweep and stage-A enqueue sweep
            xt_sched = {0: 1, 2: 2, 6: 3}       # sweep -> batch to DMA
            sa_sched = {1: 1, 5: 2, 9: 3}       # sweep -> batch to enqueue

            emit_score_pair(0, 0)
            prev = None
            for s in range(NS):
                for jb in range(NJ):
                    if s in xt_sched:
                        # 2 DMA pieces per step -> 32 pieces over the sweep,
                        # interleaved with proj out-DMAs on the Sync queue
                        if jb == 0:
                            alloc_xt(xt_sched[s])
                        emit_xt_piece(xt_sched[s], 2 * jb)
                        emit_xt_piece(xt_sched[s], 2 * jb + 1)
                    if jb == 0:
                        if s in sa_sched:
                            filler.extend(make_stage_a_units(sa_sched[s]))
                    emit_exp(s, jb)
                    if not (s == NS - 1 and jb == NJ - 1):
                        ns, njb = (s, jb + 1) if jb + 1 < NJ else (s + 1, 0)
                        emit_score_pair(ns, njb)
                    if prev is not None:
                        emit_attnv(*prev)
                        emit_esum(*prev)
                    prev = (s, jb)
                    # sweep 0: ACT is still ramping, so drain stage-A(0)
                    # aggressively (needed for emission-order correctness of
                    # kt/v2 reads within sweep 0)
                    budget = 3000 if s == 0 else 900
                    if jb == 1 and s >= 1:
                        emit_norm_a(s - 1)
                        budget = 500
                    elif jb == 3 and s >= 1:
                        emit_norm_b(s - 1)
                        budget = 500
                    pop_filler(budget)

            # tail: last attnV/esum, last norm, drain filler, then the
            # final sweep's projection from the freed attnV/score banks
            emit_attnv(*prev)
            emit_esum(*prev)
            emit_norm_a(NS - 1)
            emit_norm_b(NS - 1)
            pop_filler(10 ** 9)
            for rr in range(4):
                for n2 in range(2):
                    proj_unit(NS - 1, rr, n2, pool=ps_av, tag="av")

    nc.compile()
    return nc


def _in_maps(x, w_qkv, w_proj):
    x2 = np.ascontiguousarray(x.reshape(R, D).T)          # (D, R)
    xbf = x2.astype(ml_dtypes.bfloat16)
    Wq = w_qkv.reshape(D, 3, H, d)
    Wp = w_proj.reshape(H, d, D)
    maps = []
    for c in range(NCORES):
        hs = slice(c * HPC, (c + 1) * HPC)
        # per-core qkv weight shard, columns ordered (qkv, head, d)
        w_shard = np.ascontiguousarray(Wq[:, :, hs, :]).reshape(D, 3 * HPC * d)
        # pre-tile: [p, kb*384 + m*128 + col] = w_shard[kb*128+p, m*128+col]
        wq_pre = np.ascontiguousarray(
            w_shard.reshape(8, 128, 3, 128).transpose(1, 0, 2, 3)
        ).reshape(128, 3072)
        wp_shard = np.ascontiguousarray(Wp[hs]).reshape(HPC * d, D)
        maps.append({
            "xT": xbf,
            "wqkv": np.ascontiguousarray(wq_pre).astype(ml_dtypes.bfloat16),
            "wproj": wp_shard.astype(ml_dtypes.bfloat16),
        })
    return maps


def get_nc():
    if "nc" not in _CACHE:
        _CACHE["nc"] = _build()
    return _CACHE["nc"]


def kernel(x, w_qkv, w_proj, b_proj):
    x = np.asarray(x)
    w_qkv = np.asarray(w_qkv)
    w_proj = np.asarray(w_proj)
    b_proj = np.asarray(b_proj)
    nc = get_nc()
    maps = _in_maps(x, w_qkv, w_proj)
    res = bass_utils.run_bass_kernel_spmd(nc, maps, core_ids=list(range(NCORES)))
    acc = np.zeros((R, D), dtype=np.float64)
    for r in res.results:
        acc += r["out"].astype(np.float64)
    acc += b_proj.astype(np.float64)
    return acc.reshape(B, P, D).astype(np.float32)
